# revision 84
# baseline (speedup 1.0000x reference)
"""Trainium2 Bass kernel for the UR5e reflected-mass cost function.

Math (per sample n of 131072 = 2048 b x 64 h):
  q = x[b,h,6:11], hand = x[b,h,19:22]  (q6 provably unused)
  cost = |d|^2 / (vd^T M^-1 vd),  out[b] = -sum_h cost

Key structural choices (vs. the naive world-frame FK chain):
  * All geometry in FRAME-1 LOCAL coordinates: z0=[0,1,0], z1=z2=z3=[0,0,1]
    are constants, so 19 of 21 Jacobian columns are sign-permutations of
    dp = p_{i+1}-p_j (no cross-product muls); M is q1-free; q1 enters only
    through rotating `hand` (4 muls).  J(0,0)=J(3,3)=J(4,4)=J(5,5)=0.
  * Sum-angle parameterization (q2, q2+q3, q2+q3+q4): 10 trig values.
    One range reduction per angle; cos(r) = Sin LUT(-|r| + pi/2) since cos
    is even -- LUT argument stays inside [-pi/2, pi/2].
  * dp vectors via telescoping link sums with symbolic coefficient/sign
    tracking (items), so structural zeros/constants cancel exactly.
  * M solved with LDL^T (no sqrt on the critical path; the pre-division
    value doubles as L*d).
  * fp16 tiles everywhere except the magic-constant range-reduction ops
    (fp16 overflows) -- DVE runs 2x on fp16.  Validated ~4.7e-3 max batch
    relative error vs the 2e-2 gate.
  * Per-node engine assignment (DVE/ACT/Pool) + HEFT-style list scheduling
    with same-engine-chain latency elision; Pool only runs TensorTensor
    (TensorScalarPtr is not in the TRN2 Pool ISA).
  * Host packs the 8 needed channels -> [N,8] f32; the stage DMA is split
    across both HWDGE queues (SP + Activation).

Each per-sample scalar is a [128,128] SBUF tile (16384 samples/core,
8 cores data-parallel over b).  ~284 instructions; TimelineSim 38.5us.
"""

import math
import numpy as np

# ----------------------------------------------------------------------------
# symbolic scalar DAG
# ----------------------------------------------------------------------------

PI = math.pi
DH_A = [0.0, -0.425, -0.3922, 0.0, 0.0, 0.0]
DH_D = [0.1625, 0.0, 0.0, 0.1333, 0.0997, 0.0996]
# exact integer cos/sin of the alpha angles [pi/2, 0, 0, pi/2, -pi/2, 0]
CA = [0, 1, 1, 0, 0, 1]
SA = [1, 0, 0, 1, -1, 0]
MASS = [3.761, 8.058, 2.846, 1.37, 1.3, 0.365]
ROTOR = 0.1


class Expr:
    __slots__ = ("op", "args", "c", "id", "users", "engine", "fused_into",
                 "slot", "order", "half")

    def __init__(self, op, args=(), c=None, i=0):
        self.op = op
        self.args = args
        self.c = c
        self.id = i
        self.users = []          # list of consumer Exprs
        self.engine = None       # 'dve' | 'act' | 'gps' | None(folded)
        self.fused_into = None   # consumer that absorbed this node
        self.slot = None
        self.order = None


class Graph:
    def __init__(self):
        self.nodes = []
        self.cse = {}

    def _mk(self, op, args=(), c=None):
        key = (op, tuple(a.id for a in args), c)
        n = self.cse.get(key)
        if n is None:
            n = Expr(op, args, c, len(self.nodes))
            self.nodes.append(n)
            self.cse[key] = n
        return n

    # ---- builders with simplification ----
    def C(self, v):
        return self._mk("const", c=float(v))

    def IN(self, ch):
        return self._mk("in", c=ch)

    def add(self, x, y):
        if x.op == "const" and y.op == "const":
            return self.C(x.c + y.c)
        if x.op == "const":
            x, y = y, x
        if y.op == "const":
            if y.c == 0.0:
                return x
            return self._mk("cadd", (x,), y.c)
        if x.op == "cmul" and x.c == -1.0:
            return self.sub(y, x.args[0])
        if y.op == "cmul" and y.c == -1.0:
            return self.sub(x, y.args[0])
        a, b = (x, y) if x.id <= y.id else (y, x)
        return self._mk("add", (a, b))

    def sub(self, x, y):
        if x.op == "const" and y.op == "const":
            return self.C(x.c - y.c)
        if y.op == "const":
            if y.c == 0.0:
                return x
            return self._mk("cadd", (x,), -y.c)
        if y.op == "cmul" and y.c == -1.0:
            return self.add(x, y.args[0])
        if x.op == "const" and x.c == 0.0:
            return self.cmul(-1.0, y)
        if x is y:
            return self.C(0.0)
        return self._mk("sub", (x, y))

    def cmul(self, c, x):
        c = float(c)
        if x.op == "const":
            return self.C(c * x.c)
        if c == 0.0:
            return self.C(0.0)
        if c == 1.0:
            return x
        if x.op == "cmul":
            return self.cmul(c * x.c, x.args[0])
        return self._mk("cmul", (x,), c)

    def _factors(self, x, out, depth=0):
        # flatten mul/square trees into leaf factors (canonical monomials)
        if depth < 4 and x.op == "mul":
            self._factors(x.args[0], out, depth + 1)
            self._factors(x.args[1], out, depth + 1)
        elif depth < 4 and x.op == "square":
            self._factors(x.args[0], out, depth + 1)
            self._factors(x.args[0], out, depth + 1)
        else:
            out.append(x)

    def mul(self, x, y):
        if x.op == "const":
            return self.cmul(x.c, y)
        if y.op == "const":
            return self.cmul(y.c, x)
        if x.op == "cmul" and y.op == "cmul":
            return self.cmul(x.c * y.c, self.mul(x.args[0], y.args[0]))
        if x.op == "cmul":
            return self.cmul(x.c, self.mul(x.args[0], y))
        if y.op == "cmul":
            return self.cmul(y.c, self.mul(x, y.args[0]))
        if x is y:
            return self._mk("square", (x,))
        a, b = (x, y) if x.id <= y.id else (y, x)
        return self._mk("mul", (a, b))

    def sinsb(self, x, scale, bias):
        # sin(scale*x + bias)
        return self._mk("sin", (x,), (float(scale), float(bias)))

    def ts2(self, x, s1, op0, s2, op1):
        # (x op0 s1) op1 s2  — one DVE tensor_scalar with two fused scalar ops
        return self._mk("ts2", (x,), (float(s1), op0, float(s2), op1))

    def trig(self, q, phase):
        """sin(q + phase) with range reduction to [-pi,pi): HW Sin LUT is
        only accurate near the principal range.  k = round((q+phase)/2pi)
        via the float magic-number trick; r0 = q - 2pi*k; sin(r0 + phase)
        with phase as activation bias."""
        MAGIC = 12582912.0  # 1.5 * 2**23: adding then subtracting rounds f32
        inv2pi = 1.0 / (2.0 * PI)
        if phase == 0.0:
            t1 = self.ts2(q, inv2pi, "mult", MAGIC, "add")
            k = self._mk("cadd", (t1,), -MAGIC)
        else:
            # phase/2pi must be added BEFORE the magic add (it would be
            # absorbed: ulp(MAGIC) = 1.0)
            t0 = self.ts2(q, inv2pi, "mult", phase * inv2pi, "add")
            t1 = self._mk("cadd", (t0,), MAGIC)
            k = self._mk("cadd", (t1,), -MAGIC)
        r0 = self.add(self.cmul(-2.0 * PI, k), q)  # fuses to one STT
        return self._mk("sin", (r0,), (1.0, float(phase)))

    def sqrt_(self, x):
        return self._mk("sqrt", (x,))

    def div(self, x, y):
        if x.op == "const" and x.c == 0.0:
            return self.C(0.0)
        if y.op == "const":
            return self.cmul(1.0 / y.c, x)
        return self._mk("div", (x, y))

    def recip(self, x):
        return self._mk("recip", (x,))

    def dot3(self, u, v):
        t = [self.mul(u[i], v[i]) for i in range(3)]
        return self.add(self.add(t[0], t[1]), t[2])

    def cross(self, a, b):
        return [
            self.sub(self.mul(a[1], b[2]), self.mul(a[2], b[1])),
            self.sub(self.mul(a[2], b[0]), self.mul(a[0], b[2])),
            self.sub(self.mul(a[0], b[1]), self.mul(a[1], b[0])),
        ]


# ---- signed/const-tracked scalar terms for the local-frame formulation ----
# item := (coef: float, expr: Expr | None); value = coef * (expr or 1).
# coef == 0.0 encodes the zero item.

def it_mul(g, a, b):
    ca, ea = a
    cb, eb = b
    c = ca * cb
    if c == 0.0:
        return (0.0, None)
    if ea is None:
        return (c, eb)
    if eb is None:
        return (c, ea)
    return (c, g.mul(ea, eb))


def it_combine(g, items):
    """Sum of items -> Expr. Folds constants; merges equal-expr terms."""
    const = 0.0
    terms = {}  # expr id -> [coef, expr]
    for c, e in items:
        if c == 0.0:
            continue
        if e is None:
            const += c
        elif e.op == "const":
            const += c * e.c
        elif e.id in terms:
            terms[e.id][0] += c
        else:
            terms[e.id] = [c, e]
    tl = [(c, e) for c, e in terms.values() if c != 0.0]
    if not tl:
        return g.C(const)
    # unit-coef terms first (plain add/sub), then cmul terms (STT-fusable);
    # negative coefs emitted as sub(acc, cmul(|c|, e)) for CSE canonicality
    tl.sort(key=lambda t: (abs(t[0]) != 1.0, t[0] < 0.0))
    acc = None
    for c, e in tl:
        if acc is None:
            acc = e if c == 1.0 else g.cmul(c, e)
        elif c == 1.0:
            acc = g.add(acc, e)
        elif c == -1.0:
            acc = g.sub(acc, e)
        elif c < 0.0:
            acc = g.sub(acc, g.cmul(-c, e))
        else:
            acc = g.add(acc, g.cmul(c, e))
    if const != 0.0:
        acc = g.add(acc, g.C(const))
    return acc


def sv_cross(g, a, b):
    def m(i, j):
        return it_mul(g, a[i], b[j])
    def sub2(x, y):
        return [x, (-y[0], y[1])]
    return [
        sub2(m(1, 2), m(2, 1)),
        sub2(m(2, 0), m(0, 2)),
        sub2(m(0, 1), m(1, 0)),
    ]  # each comp = list of items (un-combined)


def build_graph():
    """Frame-1 local-coordinate formulation.

    local coords: world = Rz(q1) * RotX(pi/2) * local + [0,0,d1].
    In this frame z0=[0,1,0], z1=z2=z3=[0,0,1] are constants; M is q1-free.
    Returns (graph, cost_neg_node). cost_neg = -cost per sample.
    """
    g = Graph()
    q1, q2, q3, q4, q5 = (g.IN(i) for i in range(5))
    hx, hy, hz = (g.IN(5 + c) for c in range(3))
    q23 = g.add(q2, q3)
    q234 = g.add(q23, q4)

    if COS_SHARED:
        # one range reduction per angle; since cos is even,
        # cos(r) = sin(pi/2 - |r|) keeps the Sin LUT arg in [-pi/2, pi/2].
        MOD_RR = _os.environ.get("KERNEL_MODRR", "0") == "1"  # mod fails walrus ISA check

        def sc_pair(q):
            if MOD_RR:
                # rp = (q + 3pi) mod 2pi in ONE ts2 op (dividend stays
                # positive for |q| < 3pi, so fmod == floored mod); the -pi
                # recentering folds into the ACT bias of Sin and Abs.
                rp = g.ts2(q, 3.0 * PI, "add", 2.0 * PI, "mod")
                ra = g._mk("abs", (rp,), -PI)
                return (g._mk("sin", (rp,), (1.0, -PI)),
                        g._mk("sin", (ra,), (-1.0, PI / 2)))
            MAGIC = 12582912.0
            inv2pi = 1.0 / (2.0 * PI)
            t1 = g.ts2(q, inv2pi, "mult", MAGIC, "add")
            k = g._mk("cadd", (t1,), -MAGIC)
            r0 = g.add(g.cmul(-2.0 * PI, k), q)
            ra = g._mk("abs", (r0,))
            return (g._mk("sin", (r0,), (1.0, 0.0)),
                    g._mk("sin", (ra,), (-1.0, PI / 2)))
        s1, c1 = sc_pair(q1)
        s2, c2 = sc_pair(q2)
        s23, c23 = sc_pair(q23)
        s234, c234 = sc_pair(q234)
        s5, c5 = sc_pair(q5)
    else:
        s1, c1 = g.trig(q1, 0.0), g.trig(q1, PI / 2)
        s2, c2 = g.trig(q2, 0.0), g.trig(q2, PI / 2)
        s23, c23 = g.trig(q23, 0.0), g.trig(q23, PI / 2)
        s234, c234 = g.trig(q234, 0.0), g.trig(q234, PI / 2)
        s5, c5 = g.trig(q5, 0.0), g.trig(q5, PI / 2)

    d1, a2, a3, d4, d5, d6 = 0.1625, -0.425, -0.3922, 0.1333, 0.0997, 0.0996
    m = MASS

    E = lambda e: (1.0, e)      # unit item from expr
    K = lambda c: (float(c), None)  # const item
    Z = (0.0, None)

    # local frame geometry (scalar Exprs)
    p2x = g.cmul(a2, c2)
    p2y = g.cmul(a2, s2)
    p3x = g.add(p2x, g.cmul(a3, c23))
    p3y = g.add(p2y, g.cmul(a3, s23))
    p5x = g.add(p3x, g.cmul(d5, s234))
    p5y = g.sub(p3y, g.cmul(d5, c234))
    t1 = g.mul(s5, c234)   # -z5x
    t2 = g.mul(s5, s234)   # -z5y
    p6x = g.sub(p5x, g.cmul(d6, t1))
    p6y = g.sub(p5y, g.cmul(d6, t2))
    p6z = g.add(g.cmul(d6, c5), g.C(d4))   # d4 + d6*c5

    # link vectors link_t = p_{t+1} - p_t as item-lists per component
    links = [
        [[], [K(d1)], []],                             # p1 - p0
        [[E(p2x)], [E(p2y)], []],                      # p2 - p1
        [[(a3, c23)], [(a3, s23)], []],                # p3 - p2
        [[], [], [K(d4)]],                             # p4 - p3
        [[(d5, s234)], [(-d5, c234)], []],             # p5 - p4
        [[(-d6, t1)], [(-d6, t2)], [(d6, c5)]],        # p6 - p5
    ]
    zs = [
        [[], [K(1.0)], []],                            # z0
        [[], [], [K(1.0)]],                            # z1 = z2 = z3
        [[], [], [K(1.0)]],
        [[], [], [K(1.0)]],
        [[E(s234)], [(-1.0, c234)], []],               # z4
        [[(-1.0, t1)], [(-1.0, t2)], [E(c5)]],         # z5
    ]

    def merge(items):
        """Merge equal-expr terms, drop zeros. Returns item list."""
        const = 0.0
        terms = {}
        for c, e in items:
            if c == 0.0:
                continue
            if e is None:
                const += c
            elif e.id in terms:
                terms[e.id][0] += c
            else:
                terms[e.id] = [c, e]
        out = [(c, e) for c, e in terms.values() if c != 0.0]
        if const != 0.0:
            out.append((const, None))
        return out

    def materialize(items):
        """item list -> single item (combining into one Expr if needed)."""
        items = merge(items)
        if not items:
            return (0.0, None)
        if len(items) == 1:
            return items[0]
        return (1.0, it_combine(g, items))

    # dp[(i,j)] = p_{i+1} - p_j, each component materialized to one item
    dp = {}
    for i in range(6):
        for j in range(i + 1):
            comps = []
            for c in range(3):
                acc = []
                for t in range(j, i + 1):
                    acc.extend(links[t][c])
                comps.append(materialize(acc))
            dp[(i, j)] = comps

    def cross_mat(zv, dv):
        """cross of two single-item svecs -> materialized single-item svec."""
        def m(i, jj):
            return it_mul(g, zv[i], dv[jj])
        return [
            materialize([m(1, 2), (lambda x: (-x[0], x[1]))(m(2, 1))]),
            materialize([m(2, 0), (lambda x: (-x[0], x[1]))(m(0, 2))]),
            materialize([m(0, 1), (lambda x: (-x[0], x[1]))(m(1, 0))]),
        ]

    # J[(i,j)] = z_j x dp_ij, materialized per component
    Jm = {}
    for i in range(6):
        for j in range(i + 1):
            zv = [materialize(zc) for zc in zs[j]]
            Jm[(i, j)] = cross_mat(zv, dp[(i, j)])

    def dot_items(A, B):
        return [it_mul(g, A[c], B[c]) for c in range(3)]

    # mass matrix upper triangle: M[j,k] = sum_i m_i J_ij . J_ik (+ rotor)
    M = {}
    for jj in range(6):
        for kk in range(jj, 6):
            items = []
            for i in range(kk, 6):
                for c, e in dot_items(Jm[(i, jj)], Jm[(i, kk)]):
                    items.append((c * m[i], e))
            if jj == kk:
                items.append(K(ROTOR))
            M[(jj, kk)] = it_combine(g, items)

    # hand in local frame: h = (c1*hx + s1*hy, hz - d1, s1*hx - c1*hy)
    dx = it_combine(g, [E(g.mul(c1, hx)), E(g.mul(s1, hy)), (-1.0, p6x)])
    dy = it_combine(g, [E(hz), K(-d1), (-1.0, p6y)])
    dz = it_combine(g, [E(g.mul(s1, hx)), (-1.0, g.mul(c1, hy)), (-1.0, p6z)])
    dv = [E(dx), E(dy), E(dz)]
    n2 = it_combine(g, merge(dot_items(dv, dv)))

    # vd_j = J_5j . d   (vd_5 == 0 structurally)
    vd = [it_combine(g, merge(dot_items(Jm[(5, j)], dv))) for j in range(6)]

    # LDL^T: M = L D L^T (L unit lower). W[k,j] = L[k,j]*d_j comes free as
    # the pre-division value, and no sqrt appears on the critical path.
    USE_DIV = _os.environ.get("KERNEL_DIV", "0") == "1"  # divide fails walrus ISA check
    L, W = {}, {}
    dds, dinv = [], []
    for jc in range(6):
        dd = M[(jc, jc)]
        for t in range(jc):
            dd = g.sub(dd, g.mul(L[(jc, t)], W[(jc, t)]))
        dds.append(dd)
        di = None if USE_DIV else g.recip(dd)
        dinv.append(di)
        for kk in range(jc + 1, 6):
            a = M[(jc, kk)]
            for t in range(jc):
                a = g.sub(a, g.mul(L[(kk, t)], W[(jc, t)]))
            W[(kk, jc)] = a
            L[(kk, jc)] = g.div(a, dd) if USE_DIV else g.mul(a, di)

    # forward solve L z = vd (unit diagonal); s = sum z_j^2 / d_j
    z = []
    for j in range(6):
        a = vd[j]
        for t in range(j):
            a = g.sub(a, g.mul(L[(j, t)], z[t]))
        z.append(a)
    sacc = None
    for j in range(6):
        zj2 = g.mul(z[j], z[j])
        t = g.div(zj2, dds[j]) if USE_DIV else g.mul(zj2, dinv[j])
        sacc = t if sacc is None else g.add(sacc, t)
    # cost_neg = -n2 / s
    if USE_DIV:
        cost_neg = g.cmul(-1.0, g.div(n2, sacc))
    else:
        cost_neg = g.mul(g.cmul(-1.0, g.recip(sacc)), n2)
    return g, cost_neg


# ----------------------------------------------------------------------------
# numpy evaluation of the DAG (for validation in test.py)
# ----------------------------------------------------------------------------

def eval_numpy(g, root, chans):
    """chans: dict ch -> np array [N]. Evaluates all nodes; returns root val."""
    val = {}
    for n in g.nodes:
        if n.op == "const":
            val[n.id] = np.float32(n.c)
        elif n.op == "in":
            val[n.id] = chans[n.c]
        elif n.op == "add":
            val[n.id] = val[n.args[0].id] + val[n.args[1].id]
        elif n.op == "sub":
            val[n.id] = val[n.args[0].id] - val[n.args[1].id]
        elif n.op == "mul":
            val[n.id] = val[n.args[0].id] * val[n.args[1].id]
        elif n.op == "square":
            val[n.id] = val[n.args[0].id] * val[n.args[0].id]
        elif n.op == "cmul":
            val[n.id] = np.float32(n.c) * val[n.args[0].id]
        elif n.op == "cadd":
            val[n.id] = val[n.args[0].id] + np.float32(n.c)
        elif n.op == "sin":
            sc, b = n.c
            val[n.id] = np.sin(np.float32(sc) * val[n.args[0].id] + np.float32(b))
        elif n.op == "ts2":
            s1, op0, s2, op1 = n.c
            v = val[n.args[0].id]
            for s_, o_ in ((s1, op0), (s2, op1)):
                if o_ == "mult":
                    v = v * np.float32(s_)
                elif o_ == "mod":
                    v = np.mod(v, np.float32(s_))
                else:
                    v = v + np.float32(s_)
            val[n.id] = v
        elif n.op == "sqrt":
            val[n.id] = np.sqrt(val[n.args[0].id])
        elif n.op == "abs":
            val[n.id] = np.abs(val[n.args[0].id] + np.float32(n.c or 0.0))
        elif n.op == "recip":
            val[n.id] = np.float32(1.0) / val[n.args[0].id]
        elif n.op == "div":
            val[n.id] = val[n.args[0].id] / val[n.args[1].id]
        else:
            raise ValueError(n.op)
        if n.op != "const":
            val[n.id] = val[n.id].astype(np.float32)
    return val[root.id]


def ref_numpy(x):
    """Full-pipeline numpy reference using the DAG; x [B,H,26] -> [B]."""
    B, H, Cc = x.shape
    N = B * H
    flat = x.reshape(N, Cc)[:, SRC_CHANS].astype(np.float32)
    g, root = build_graph()
    chans = {ch: flat[:, ch] for ch in range(CH)}
    cn = eval_numpy(g, root, chans)
    return cn.reshape(B, H).sum(axis=1)


# ----------------------------------------------------------------------------
# planning: use counts, fusion, engine assignment, slot allocation
# ----------------------------------------------------------------------------

MAGIC_LIM = 6e4  # range-reduction magic constants must stay f32


def _kind(n):
    if n.op in ("sin", "sqrt"):
        return "act"
    if n.op == "recip":
        return "recip"
    if n.op in ("cadd", "ts2"):
        return "ts"
    if n.op in ("abs", "div"):
        return "tt"
    if n.op == "cmul":
        if n.args[0].fused_into is n:
            return "stt"
        return "ts"
    if n.op in ("add", "sub") and isinstance(n.c, tuple):
        return "stt"
    if n.op in ("add", "sub", "mul", "square"):
        return "tt"
    raise ValueError(n.op)


def plan(g, root, gps_frac=1.0):
    """Fusion + dtype + engine assignment + list scheduling.

    Returns (emit, load) where emit is the scheduled list of nodes with
    .engine set; each node also gets .half (fp16 output tile).
    """
    reach = set()
    stack = [root]
    while stack:
        n = stack.pop()
        if n.id in reach:
            continue
        reach.add(n.id)
        stack.extend(n.args)
    for n in g.nodes:
        n.users = []
    order = [n for n in g.nodes if n.id in reach]
    for n in order:
        for a in n.args:
            a.users.append(n)

    # fusion: add/sub(x, cmul(c,y)) -> STT ; cmul(c, mul/square(x)) -> STT
    import os as _osf
    if _osf.environ.get("KERNEL_FUSE", "1") == "0":
        order_fuse = []
    else:
        order_fuse = order
    for n in order_fuse:
        if n.op in ("add", "sub"):
            for k, a in enumerate(n.args):
                if a.op == "cmul" and len(a.users) == 1 and a.fused_into is None \
                        and a.args[0].fused_into is None \
                        and a.args[0].op not in ("const",):
                    n.c = ("stt_cmul", k, a.c)
                    a.fused_into = n
                    break
        elif n.op == "cmul" and n.fused_into is None:
            a = n.args[0]
            if a.op in ("mul", "square") and len(a.users) == 1 \
                    and a.fused_into is None \
                    and all(aa.fused_into is None for aa in a.args):
                a.fused_into = n

    # ---- trig block: the 5 range-reduction chains + wide-ACT sin/abs/cos.
    # These nodes are emitted manually (wide [128, 5*FD] ACT ops); exclude
    # them from generic scheduling, but preset their ready times.
    trig_info = {"angles": [], "presched": [], "ids": set()}
    if COS_SHARED and _osf.environ.get("KERNEL_WIDETRIG", "0") == "1":
        sinA = [n for n in order if n.op == "sin" and n.c == (1.0, 0.0)]
        for s in sinA:
            r0 = s.args[0]
            absn = next(u for u in r0.users if u.op == "abs")
            cosn = next(u for u in absn.users if u.op == "sin")
            cm = next(a for a in r0.args if a.op == "cmul")
            qsrc = next(a for a in r0.args if a is not cm)
            k = cm.args[0]
            t1 = k.args[0]
            rec = {"q": qsrc, "t1": t1, "k": k, "r0": r0,
                   "sin": s, "abs": absn, "cos": cosn}
            trig_info["angles"].append(rec)
            trig_info["ids"].update(x.id for x in (t1, k, r0, s, absn, cosn))
        # angle-sum adds (q23, q234) feeding the chains
        for rec in trig_info["angles"]:
            q = rec["q"]
            if q.op == "add" and q.id not in trig_info["ids"]:
                trig_info["ids"].add(q.id)
        pres = [n for n in order if n.id in trig_info["ids"]
                and n.op == "add" and not isinstance(n.c, tuple)]
        trig_info["presched"] = sorted(pres, key=lambda n: n.id)

    emit_nodes = [n for n in order if n.op not in ("const", "in")
                  and n.fused_into is None and n.id not in trig_info["ids"]]

    # ---- dtype assignment: fp16 everywhere except range-reduction magic
    # and the f32-only reciprocal (plus its inputs) ----
    for n in order:
        n.half = True
        if n.op == "ts2" and (abs(n.c[2]) > MAGIC_LIM or n.c[3] == "mod"):
            n.half = False  # magic consts overflow fp16; mod result needs f32
        if n.op == "cadd" and abs(n.c) > MAGIC_LIM:
            n.half = False
        if n.op == "recip":
            n.half = False
    if _osf.environ.get("KERNEL_RECIP2", "1") != "1":
        for n in order:
            if n.op == "recip":
                for a in n.args:
                    a.half = False  # reciprocal_approx_fast needs f32 input

    def real_args(n):
        out = []
        for a in n.args:
            if a.fused_into is n:
                out.extend(a.args)
            else:
                out.append(a)
        return out

    _NEWTBL = _osf.environ.get("KERNEL_COSTTBL", "old") == "new"

    def cost(n, e):
        k = _kind(n)
        if _NEWTBL:
            # effective per-op costs measured from TimelineSim (w/ issue ovh)
            if e == "act":
                return 337.0
            pure16 = n.half and all(a.op != "in" and a.half
                                    for a in real_args(n))
            if e == "dve":
                if k == "tt":
                    return 149.0 if pure16 else 216.0
                if k == "ts":
                    return 116.0 if pure16 else 149.0
                return 216.0
            return 371.0
        if e == "act":
            return 292.0
        pure16 = n.half and all(a.op != "in" and a.half for a in real_args(n))
        if e == "dve":
            if k == "tt":
                return 127.0 if pure16 else 194.0
            if k == "ts":
                return 94.0 if pure16 else 127.0
            return 194.0  # stt / recip
        return 349.0  # gps (Pool)

    def engines(n):
        # Pool(TRN2) only supports TensorTensor among our op kinds
        k = _kind(n)
        if n.op in ("sin", "sqrt"):
            return ("act",)
        if n.op == "recip":
            return ("dve",)
        if k == "stt" or n.op == "ts2":
            return ("dve",)
        if n.op == "abs":
            return ("act",)
        if n.op == "div":
            return ("dve",)
        if n.op == "cadd":
            return ("dve",) if not n.half else ("dve", "act")
        if n.op == "cmul":
            return ("dve", "act")
        if n.op == "square":
            return ("dve", "gps", "act")
        return ("dve", "gps")  # tt: add/sub/mul

    # ---- priorities: longest path to sink ----
    import os as _osl
    _LS = float(_osl.environ.get("KERNEL_LATS", "1.0"))
    LAT = {"dve": 230.0 * _LS, "act": 285.0 * _LS, "gps": 161.0 * _LS}
    best = {n.id: min(cost(n, e) for e in engines(n)) for n in emit_nodes}
    pr = {}
    emit_set = {n.id for n in emit_nodes}

    def consumers(n):
        out = []
        for u in n.users:
            if u.fused_into is not None:
                out.extend(uu for uu in u.users if uu.id in emit_set)
            elif u.id in emit_set:
                out.append(u)
        return out

    import random as _rnd
    _seed = int(_osl.environ.get("KERNEL_SEED", "7"))
    _jit = _rnd.Random(_seed) if _seed else None
    # forward depth (longest path from sources) for priority blending
    depth = {}
    for n in order:
        if n.id not in emit_set:
            continue
        dmax = 0.0
        for a in n.args:
            if a.fused_into is n:
                for aa in a.args:
                    if aa.id in depth:
                        dmax = max(dmax, depth[aa.id])
            elif a.id in depth:
                dmax = max(dmax, depth[a.id])
        depth[n.id] = dmax + best[n.id] + 150.0
    _BETA = float(_osl.environ.get("KERNEL_BETA", "1.0"))
    _amp = float(_osl.environ.get("KERNEL_AMP", "0.08"))
    _PLAT = float(_osl.environ.get("KERNEL_PLAT", "150"))
    for n in reversed(order):
        if n.id not in emit_set:
            continue
        ucs = consumers(n)
        tail = max((pr[u.id] + _PLAT for u in ucs), default=0.0)
        pr[n.id] = best[n.id] + tail
        if _jit is not None:
            pr[n.id] *= 1.0 + _amp * (_jit.random() - 0.5)
    if _BETA != 1.0:
        dm = max(depth.values()) if depth else 1.0
        for nid in pr:
            # nodes deep from the source get de-prioritized by (1-beta)
            pr[nid] = _BETA * pr[nid] + (1.0 - _BETA) * (dm - depth[nid])

    # ---- list scheduling ----
    import heapq
    import os as _os2
    T_IN = float(_os2.environ.get("KERNEL_TIN", "3200"))
    ALPHA = float(_os2.environ.get("KERNEL_ALPHA", "0"))
    T_TRG = float(_os2.environ.get("KERNEL_TTRIG", "2300"))
    _EJIT = float(_os2.environ.get("KERNEL_EJIT", "0"))
    preset_rdy = {}
    for rec in trig_info["angles"]:
        preset_rdy[rec["cos"].id] = T_IN + T_TRG + 800.0
        preset_rdy[rec["sin"].id] = T_IN + T_TRG + 1600.0
    ndeps = {}
    dep_nodes = {}
    for n in emit_nodes:
        deps = [a for a in real_args(n) if a.id in emit_set]
        dep_nodes[n.id] = deps
        ndeps[n.id] = len(set(a.id for a in deps))
    pending = dict(ndeps)
    done_t = {}
    eng_free = {"dve": 0.0, "act": 0.0, "gps": 0.0}
    load = {"dve": 0.0, "act": 0.0, "gps": 0.0}
    heap = []
    for n in emit_nodes:
        if pending[n.id] == 0:
            heapq.heappush(heap, (-pr[n.id], n.id))
    sched = []
    dec_done = set()
    nodes_by_id = {n.id: n for n in emit_nodes}
    while heap:
        _, nid = heapq.heappop(heap)
        n = nodes_by_id[nid]
        rdy0 = T_IN if any(a.op == "in" for a in real_args(n)) else 0.0
        for a in real_args(n):
            if a.id in preset_rdy:
                rdy0 = max(rdy0, preset_rdy[a.id])
        best_e, best_fin = None, None
        for e in engines(n):
            rdy = rdy0
            for a in dep_nodes[nid]:
                fin_a, e_a = done_t[a.id]
                # same-engine consumers are ordered by program order: no
                # cross-engine semaphore latency
                rdy = max(rdy, fin_a if e_a == e else fin_a + LAT[e_a])
            fin = max(rdy, eng_free[e]) + cost(n, e)
            score = fin + ALPHA * load[e]
            if _jit is not None and _EJIT > 0.0:
                score *= 1.0 + _EJIT * _jit.random()
            if best_fin is None or score < best_fin:
                best_e, best_fin, best_f = e, score, fin
        n.engine = best_e
        eng_free[best_e] = best_f
        load[best_e] += cost(n, best_e)
        done_t[nid] = (best_f, best_e)
        sched.append(n)
        # release users
        for u in consumers(n):
            key = (nid, u.id)
            if key in dec_done:
                continue
            dec_done.add(key)
            pending[u.id] -= 1
            if pending[u.id] == 0:
                heapq.heappush(heap, (-pr[u.id], u.id))

    for i, n in enumerate(sched):
        n.order = i
    return sched, load, trig_info


# ----------------------------------------------------------------------------
# bass emission
# ----------------------------------------------------------------------------

NCORES = 8
B_FULL, H = 2048, 64
CH = 8                                      # packed channels: q1..q5, hx,hy,hz
SRC_CHANS = [6, 7, 8, 9, 10, 19, 20, 21]
N_PER_CORE = B_FULL * H // NCORES          # 16384
P = 128
FD = N_PER_CORE // P                        # 128

import os as _os
COS_SHARED = _os.environ.get("KERNEL_COS2", "1") == "1"  # abs-folded: LUT arg stays in principal range


def _build_bass(gps_frac=1.0, repeat=1):
    import concourse.bass as bass
    from concourse.bacc import Bacc
    import concourse.mybir as mybir
    from concourse.tile import TileContext

    f32 = mybir.dt.float32
    f16 = mybir.dt.float16
    alu = mybir.AluOpType
    AF = mybir.ActivationFunctionType

    g, root = build_graph()
    emit, load, trig_info = plan(g, root, gps_frac)

    nc = Bacc()
    # register const APs needed as activation biases (non-Copy funcs)
    biases = {PI / 2}
    for n in emit:
        if n.op == "cadd" and n.engine == "act":
            biases.add(float(n.c))
        elif n.op == "sin" and n.c[1] != 0.0:
            biases.add(float(n.c[1]))
        elif n.op == "abs" and n.c:
            biases.add(float(n.c))
    for cv in sorted(biases):
        t = nc.alloc_sbuf_tensor(f"constf32-{cv}", [128, 1], f32)
        nc.gpsimd.memset(t.ap(), cv)
        nc.const_aps.aps[(f32, float(cv))] = t.ap()
    nc.all_engine_barrier()
    xs = nc.dram_tensor("xs", (N_PER_CORE, CH), f32, kind="ExternalInput")
    out = nc.dram_tensor("out", (B_FULL // NCORES,), f32, kind="ExternalOutput")

    # liveness for slot allocation
    last_use = {}
    for n in emit:
        for a in n.args:
            if a.order is not None:
                last_use[a.id] = max(last_use.get(a.id, -1), n.order)
            # fused producer's args are read by n as well
            if a.fused_into is n:
                for aa in a.args:
                    if aa.order is not None:
                        last_use[aa.id] = max(last_use.get(aa.id, -1), n.order)
    last_use[root.id] = len(emit) + 10

    with TileContext(nc) as tc:
        with tc.tile_pool(name="vals", bufs=1) as vp:
            stage = vp.tile([P, FD * CH], f32, tag="stage", bufs=2)
            src = xs.rearrange("(p q) c -> p (q c)", p=P)
            # split input load across both HWDGE queues (SP + Activation)
            if _os.environ.get("KERNEL_DMA3", "0") == "1":
                c3 = (FD // 3) * CH
                nc.sync.dma_start(stage[:, :c3], src[:, :c3])
                nc.scalar.dma_start(stage[:, c3:2 * c3], src[:, c3:2 * c3])
                nc.gpsimd.dma_start(stage[:, 2 * c3:], src[:, 2 * c3:])
            else:
                halfc = (FD // 2) * CH
                nc.sync.dma_start(stage[:, :halfc], src[:, :halfc])
                nc.scalar.dma_start(stage[:, halfc:], src[:, halfc:])
            stage3 = stage.rearrange("p (q c) -> p q c", c=CH)

            # warm up the ACT function tables while the DMA is in flight --
            # each InstLoadActFuncSet costs 1283ns and would otherwise
            # serialize right before the first Sin in the critical head
            warm = vp.tile([P, 1], f32, tag="warm", name="warm", bufs=2)
            cap = nc.const_aps.aps[(f32, PI / 2)]
            for fn_ in (AF.Sin, AF.Abs, AF.Square, AF.Sqrt):
                nc.scalar.activation(warm[:, :], cap, fn_)
            nc.scalar.add(warm[:, :], cap, PI / 2)   # Identity
            nc.scalar.mul(warm[:, :], cap, 1.0)      # Copy

            from collections import deque
            free_slots = {"h": deque(), "f": deque()}
            SLACK = int(_os.environ.get("KERNEL_SLACK", "64"))  # keep reuse distance long so WAR waits are elided
            n_slots = {"h": 0, "f": 0}
            node_tile = {}
            node_ap = {}

            def ap_of(n):
                if n.op == "in":
                    return stage3[:, :, n.c]
                if n.id in node_ap:
                    return node_ap[n.id]
                return node_tile[n.id][:, :]

            # ---- trig block: co-locate the 5 range-reduced angles in one
            # wide [P, 5*FD] tile; all sines / abs / cosines are then just
            # 3 wide ACT instructions instead of 15 narrow ones.
            angles = trig_info["angles"]
            if angles:
                A = len(angles)
                W0 = vp.tile([P, A * FD], f16, tag="trigW0", name="trigW0",
                             bufs=2)
                W1 = vp.tile([P, A * FD], f16, tag="trigW1", name="trigW1",
                             bufs=2)
                W2 = vp.tile([P, A * FD], f16, tag="trigW2", name="trigW2",
                             bufs=2)
                W3 = vp.tile([P, A * FD], f16, tag="trigW3", name="trigW3",
                             bufs=2)
                for n in trig_info["presched"]:   # q23, q234 angle sums
                    t = vp.tile([P, FD], f32, tag=f"tq{n.id}",
                                name=f"v{n.id}", bufs=2)
                    node_tile[n.id] = t
                    nc.vector.tensor_tensor(t[:, :], ap_of(n.args[0]),
                                            ap_of(n.args[1]),
                                            mybir.AluOpType.add)
                for i, rec in enumerate(angles):
                    t1 = vp.tile([P, FD], f32, tag=f"tt{i}", name=f"tt{i}",
                                 bufs=2)
                    kt = vp.tile([P, FD], f32, tag=f"tk{i}", name=f"tk{i}",
                                 bufs=2)
                    s1v, op0, s2v, op1 = rec["t1"].c
                    nc.vector.tensor_scalar(t1[:, :], ap_of(rec["q"]),
                                            float(s1v), float(s2v),
                                            getattr(mybir.AluOpType, op0),
                                            getattr(mybir.AluOpType, op1))
                    nc.vector.tensor_scalar_add(kt[:, :], t1[:, :],
                                                float(rec["k"].c))
                    _, _, cval = rec["r0"].c  # ("stt_cmul", idx, -2*pi)
                    sl = slice(i * FD, (i + 1) * FD)
                    nc.vector.scalar_tensor_tensor(
                        W0[:, sl], kt[:, :], float(cval), ap_of(rec["q"]),
                        mybir.AluOpType.mult, mybir.AluOpType.add)
                    node_ap[rec["r0"].id] = W0[:, sl]
                    node_ap[rec["sin"].id] = W1[:, sl]
                    node_ap[rec["abs"].id] = W2[:, sl]
                    node_ap[rec["cos"].id] = W3[:, sl]
                AFt = mybir.ActivationFunctionType
                # cosines feed the critical p-chain: emit Abs->Cos first
                nc.scalar.activation(W2[:, :], W0[:, :], AFt.Abs)
                nc.scalar.activation(W3[:, :], W2[:, :], AFt.Sin,
                                     bias=PI / 2, scale=-1.0)
                nc.scalar.activation(W1[:, :], W0[:, :], AFt.Sin,
                                     bias=0.0, scale=1.0)

            def alloc(n):
                dk = "h" if n.half else "f"
                if len(free_slots[dk]) > SLACK:
                    sl = free_slots[dk].popleft()
                else:
                    sl = n_slots[dk]
                    n_slots[dk] += 1
                t = vp.tile([P, FD], f16 if n.half else f32,
                            tag=f"{dk}{sl}", name=f"v{n.id}", bufs=2)
                n.slot = (dk, sl)
                node_tile[n.id] = t
                return t

            # precompute: nodes whose last use is at order i
            by_last = {}
            for nid, lu in last_use.items():
                by_last.setdefault(lu, []).append(nid)

            eng = {"dve": nc.vector, "act": nc.scalar, "gps": nc.gpsimd}
            ALU_OF = {"add": alu.add, "sub": alu.subtract, "mul": alu.mult}

            prefix = int(_os.environ.get("KERNEL_PREFIX", "0")) or len(emit)
            emit = emit[:prefix]
            for n in emit:
                ot = alloc(n)[:, :]
                e = eng[n.engine]
                if n.op == "sin":
                    sc, b = n.c
                    nc.scalar.activation(ot, ap_of(n.args[0]), AF.Sin,
                                         bias=float(b), scale=float(sc))
                elif n.op == "sqrt":
                    nc.scalar.activation(ot, ap_of(n.args[0]), AF.Sqrt)
                elif n.op == "recip":
                    if n.args[0].op not in ("const", "in") and n.args[0].half:
                        nc.vector.reciprocal(out=ot, in_=ap_of(n.args[0]))
                    else:
                        nc.vector.reciprocal_approx_fast(out=ot,
                                                         in_=ap_of(n.args[0]))
                elif n.op == "square":
                    if n.engine == "act":
                        nc.scalar.activation(ot, ap_of(n.args[0]), AF.Square)
                    else:
                        a = ap_of(n.args[0])
                        e.tensor_tensor(ot, a, a, alu.mult)
                elif n.op == "cadd":
                    if n.engine == "act":
                        nc.scalar.add(ot, ap_of(n.args[0]), float(n.c))
                    else:
                        e.tensor_scalar_add(ot, ap_of(n.args[0]), float(n.c))
                elif n.op == "abs":
                    nc.scalar.activation(ot, ap_of(n.args[0]), AF.Abs,
                                         bias=float(n.c or 0.0))
                elif n.op == "ts2":
                    s1, op0, s2, op1 = n.c
                    e.tensor_scalar(ot, ap_of(n.args[0]), float(s1), float(s2),
                                    getattr(alu, op0), getattr(alu, op1))
                elif n.op == "cmul":
                    a = n.args[0]
                    if a.fused_into is n:
                        # STT: (x * c) op y
                        if a.op == "square":
                            x = yv = a.args[0]
                        else:
                            x, yv = a.args
                        e.scalar_tensor_tensor(ot, ap_of(x), float(n.c),
                                               ap_of(yv), alu.mult, alu.mult)
                    elif n.engine == "act":
                        nc.scalar.mul(ot, ap_of(n.args[0]), float(n.c))
                    else:
                        e.tensor_scalar_mul(ot, ap_of(n.args[0]), float(n.c))
                elif n.op in ("add", "sub"):
                    if isinstance(n.c, tuple) and n.c and n.c[0] == "stt_cmul":
                        _, k, cval = n.c
                        cm = n.args[k]
                        other = n.args[1 - k]
                        x = cm.args[0]
                        if n.op == "add":
                            # (x*c) + other
                            e.scalar_tensor_tensor(ot, ap_of(x), float(cval),
                                                   ap_of(other), alu.mult, alu.add)
                        else:
                            if k == 1:
                                # other - (x*c) = (x*-c) + other
                                e.scalar_tensor_tensor(ot, ap_of(x), float(-cval),
                                                       ap_of(other), alu.mult,
                                                       alu.add)
                            else:
                                # (x*c) - other
                                e.scalar_tensor_tensor(ot, ap_of(x), float(cval),
                                                       ap_of(other), alu.mult,
                                                       alu.subtract)
                    else:
                        e.tensor_tensor(ot, ap_of(n.args[0]), ap_of(n.args[1]),
                                        ALU_OF[n.op])
                elif n.op == "mul":
                    e.tensor_tensor(ot, ap_of(n.args[0]), ap_of(n.args[1]),
                                    alu.mult)
                elif n.op == "div":
                    e.tensor_tensor(ot, ap_of(n.args[0]), ap_of(n.args[1]),
                                    alu.divide)
                else:
                    raise ValueError(n.op)

                # free slots whose last use was this node
                for nid in by_last.get(n.order, []):
                    nd = g.nodes[nid]
                    if nd.slot is not None and nd.id != root.id:
                        free_slots[nd.slot[0]].append(nd.slot[1])
                        nd.slot = None

            # epilogue: per-b sums (64-sample segments), negate already folded
            osum = vp.tile([P, 2], f32, tag="osum", bufs=2)
            croot = node_tile.get(root.id) or node_tile[emit[-1].id]
            out_r = out.rearrange("(p j) -> p j", p=P)
            if _os.environ.get("KERNEL_OSPLIT", "0") == "1":  # split out-DMA loses ~0.6us
                # first column DMAs (sync queue) while the second reduce
                # still runs; the two DMA init delays overlap
                nc.vector.tensor_reduce(osum[:, 0:1], croot[:, 0:64],
                                        mybir.AxisListType.X, alu.add)
                nc.sync.dma_start(out_r[:, 0:1], osum[:, 0:1])
                nc.vector.tensor_reduce(osum[:, 1:2], croot[:, 64:128],
                                        mybir.AxisListType.X, alu.add)
                nc.scalar.dma_start(out_r[:, 1:2], osum[:, 1:2])
            else:
                nc.vector.tensor_reduce(osum[:, 0:1], croot[:, 0:64],
                                        mybir.AxisListType.X, alu.add)
                nc.vector.tensor_reduce(osum[:, 1:2], croot[:, 64:128],
                                        mybir.AxisListType.X, alu.add)
                nc.sync.dma_start(out_r[:, :], osum[:, :])

    # run the bacc lowering passes (register allocation, wait splitting);
    # run_bass_via_pjrt serializes nc without calling finalize()
    nc.compile()
    return nc, len(emit), load, dict(n_slots)


_CACHE = {}


def kernel(x, cond, time):
    from concourse.bass_utils import run_bass_kernel_spmd

    if "nc" not in _CACHE:
        import os as _os
        nc, n_ops, load, nsl = _build_bass(gps_frac=float(_os.environ.get("KERNEL_GPS", "1.0")))
        _CACHE["nc"] = nc
    nc = _CACHE["nc"]

    xf = np.ascontiguousarray(
        np.asarray(x, dtype=np.float32).reshape(B_FULL * H, 26)[:, SRC_CHANS])
    in_maps = []
    for k in range(NCORES):
        shard = xf[k * N_PER_CORE:(k + 1) * N_PER_CORE]
        in_maps.append({"xs": np.ascontiguousarray(shard)})
    res = run_bass_kernel_spmd(nc, in_maps, core_ids=list(range(NCORES)))
    _CACHE["exec_time_ns"] = res.exec_time_ns
    _CACHE["trace"] = res.instructions_and_trace
    outs = [res.results[k]["out"] for k in range(NCORES)]
    return np.concatenate(outs).astype(np.float32)


if __name__ == "__main__":
    # quick DAG stats
    g, root = build_graph()
    emit, load, trig_info = plan(g, root)
    from collections import Counter
    print("emitted ops:", len(emit))
    print(Counter((n.engine, n.op) for n in emit))
    print("load est (us):", {k: v / 1000 for k, v in load.items()})

